# revision 33
# baseline (speedup 1.0000x reference)
"""DenseTransformerConv (GNN message passing) fused Bass/Tile kernel for Trainium2.

Sharding: 8 cores = 4 batches x 2 i-halves (data parallel; weights replicated).
Per core: b = core//2, destination-node block i in [128*(core%2), +128).

v2 design (vs v1 baseline at ~110us):
  - Edge tensor arrives from the HOST in both layouts the PE needs:
      en  [cj][128 j, 128 i, 64 de]  bf16  (j-partitioned: agg / out_v)
      eT2 [cj][128 (ii,de), 64 pr, 128 j] fp8e4m3 (de-partitioned: scores)
    -> no on-chip PE transposes, fully contiguous >=1MiB HWDGE DMAs.
  - All small tensors (weights/x/mask-bias/biases/ones) packed into ONE
    [128, 4992] bf16 buffer -> a single DMA instead of ~20.
  - No zero-fill matmuls: every PSUM accumulation group opens with start=True.
  - Scores are j-partitioned (qk batched over i); the edge-score matmul
    uses the pair-transposed fp8 tiles as 128-col stationaries (FWL-able).
  - agg is pair-batched: lhsT = en[j, (2i,64de)] (128-col stationary),
    rhs = alpha[j, (2i,8h)]; the two off-diagonal blocks are junk and the
    diagonal is extracted with 4 strided DVE copies.
  - out_v/out_e accumulate I-PARTITIONED [128 i, 8h*33]: col 33 of each head
    is a ones-column of V, so the softmax denominator falls out of the same
    matmul; normalize+skip-add are two [128,256] DVE ops. No epilogue
    transposes, no 1024-element reciprocal.
  - Scores are scaled x16 on the host (Wq,bq) so u stays in fp8 range;
    exp() applies scale=1/16.
"""

import sys

for _p in ("/opt/trn_rl_repo",):
    if _p not in sys.path:
        sys.path.append(_p)

import numpy as np
import ml_dtypes

B, N, D, DE, H, C = 4, 256, 256, 64, 8, 32
P = 128
NCORES = 8

# packed buffer column offsets (bf16 elements). Prefix PK1 holds everything
# the prologue needs so its (smaller, earlier) DMA unblocks compute sooner.
OFF_WQ, OFF_WK, OFF_XT, OFF_XTI, OFF_WETS, OFF_MB = 0, 512, 1024, 1536, 1792, 2304
PK1_COLS = 2560
OFF_WV, OFF_WS, OFF_WE, OFF_BIAS, OFF_ONES = 2560, 3072, 3584, 3840, 4864
PACK_COLS = 5376

_PROGRAM = {}


def _build_program(has_bias):
    import concourse.bass as bass
    import concourse.mybir as mybir
    import concourse.tile as tile
    from concourse.bass import ds
    from contextlib import ExitStack

    f32 = mybir.dt.float32
    bf16 = mybir.dt.bfloat16
    fp8 = mybir.dt.float8e4
    AF = mybir.ActivationFunctionType
    MUL = mybir.AluOpType.mult
    ADD = mybir.AluOpType.add

    nc = bass.Bass()

    packed = nc.declare_dram_parameter("packed", [P, PACK_COLS], bf16, isOutput=False)
    en = nc.declare_dram_parameter("en", [2, P, P * DE], bf16, isOutput=False)
    eT2 = nc.declare_dram_parameter("eT2", [2, P, 64 * P], fp8, isOutput=False)
    out = nc.declare_dram_parameter("out", [P, D], f32, isOutput=True)

    with tile.TileContext(nc) as tc, ExitStack() as ctx:
        singles = ctx.enter_context(tc.tile_pool(name="singles", bufs=1))
        fin_pool = ctx.enter_context(tc.tile_pool(name="fin", bufs=2))
        # persistent PSUM: F (skip) 1 bank, OV_i 1 bank, agg 2 banks
        f_pool = ctx.enter_context(tc.tile_pool(name="f", bufs=1, space="PSUM"))
        ov_pool = ctx.enter_context(tc.tile_pool(name="ov", bufs=1, space="PSUM"))
        agg_pool = ctx.enter_context(tc.tile_pool(name="agg", bufs=1, space="PSUM"))
        proj_ctx = ExitStack()
        proj_ps = proj_ctx.enter_context(
            tc.tile_pool(name="proj_ps", bufs=2, space="PSUM")
        )

        def mm(out_ap, lhsT, rhs, **kw):
            # every PSUM region's first writer uses start=True; order of the
            # independent regions is irrelevant -> skip sim group tracking
            kw.setdefault("skip_group_check", True)
            nc.tensor.matmul(out_ap, lhsT, rhs, **kw)

        # ---------------- small tensors in two DMAs ----------------
        # pk1 (prologue-critical prefix) first on the scalar HWDGE ring;
        # the sync HWDGE ring starts on eT2 concurrently.
        pk = singles.tile([P, PACK_COLS], bf16)
        nc.scalar.dma_start(out=pk[:, ds(0, PK1_COLS)],
                            in_=packed[:, ds(0, PK1_COLS)])
        nc.scalar.dma_start(out=pk[:, ds(PK1_COLS, PACK_COLS - PK1_COLS)],
                            in_=packed[:, ds(PK1_COLS, PACK_COLS - PK1_COLS)])

        def w_ap(base, kc, lo, n):  # weight chunk [128, n] cols lo..lo+n
            return pk[:, ds(base + kc * 256 + lo, n)]

        ones_row = pk[ds(0, 1), ds(OFF_ONES, 512)]

        def b_row(idx, lo, n):  # bias row [1, n]
            return pk[ds(0, 1), ds(OFF_BIAS + idx * 256 + lo, n)]

        # big edge DMAs: eT2 (scores) on the sync HWDGE ring — cj0 in
        # quarters so score iq0 unblocks on the first 0.25 MiB — en (agg)
        # on the scalar HWDGE ring after pk.
        eT2_sb = singles.tile([P, 2, 64 * P], fp8)
        en_sb = singles.tile([P, 2, P * DE], bf16)
        for qt in range(4):
            sl = ds(qt * 2048, 2048)
            nc.sync.dma_start(out=eT2_sb[:, 0, sl], in_=eT2[0][:, sl])
        for hf in range(2):
            sl = ds(hf * 4096, 4096)
            nc.sync.dma_start(out=eT2_sb[:, 1, sl], in_=eT2[1][:, sl])
        for cj in range(2):
            for hf in range(2):
                sl = ds(hf * 4096, 4096)
                nc.scalar.dma_start(out=en_sb[:, cj, sl], in_=en[cj][:, sl])

        # ---------------- projections ----------------
        # head-split c-partitioned (PE base-partition must be 0/32/64):
        # QTi [32 c, 8 h, 128 i], KT [32 c, 8 h, 256 j] (pre-scaled)
        QTi = singles.tile([32, H, P], bf16)
        KT = singles.tile([32, H, N], bf16)
        q_ps = proj_ps.tile([32, H, P], f32, tag="proj")
        for h in range(H):
            for kc in range(2):
                mm(q_ps[:, h, :], w_ap(OFF_WQ, kc, h * 32, 32),
                   pk[:, ds(OFF_XTI + kc * 128, 128)],
                   start=(kc == 0), stop=(kc == 1 and not has_bias))
            if has_bias:
                mm(q_ps[:, h, :], b_row(0, h * 32, 32), ones_row[:, :P],
                   start=False, stop=True)
        nc.scalar.activation(out=QTi, in_=q_ps, func=AF.Copy)
        for hh in range(2):
            k_ps = proj_ps.tile([32, 4, N], f32, tag="proj")
            for hm in range(4):
                h = hh * 4 + hm
                for kc in range(2):
                    mm(k_ps[:, hm, :], w_ap(OFF_WK, kc, h * 32, 32),
                       w_ap(OFF_XT, kc, 0, 256),
                       start=(kc == 0), stop=(kc == 1 and not has_bias))
                if has_bias:
                    mm(k_ps[:, hm, :], b_row(1, h * 32, 32), ones_row[:, :N],
                       start=False, stop=True)
            nc.scalar.activation(out=KT[:, ds(hh * 4, 4), :], in_=k_ps,
                                 func=AF.Copy)

        # V [128 j, cj, 8 h, 33]: col 32 per head = 1.0 (denominator column)
        V_sb = singles.tile([P, 2, H, 33], bf16)
        nc.vector.memset(V_sb, 1.0)  # sets the ones-columns; rest overwritten
        for cj in range(2):
            v_ps = proj_ps.tile([P, D], f32, tag="proj")
            for kc in range(2):
                mm(v_ps, w_ap(OFF_XT, kc, cj * 128, 128),
                   w_ap(OFF_WV, kc, 0, 256),
                   start=(kc == 0), stop=(kc == 1 and not has_bias))
            if has_bias:
                mm(v_ps, ones_row[:, :P], b_row(2, 0, 256),
                   start=False, stop=True)
            nc.vector.tensor_copy(
                out=V_sb[:, cj, :, 0:32],
                in_=v_ps.rearrange("p (h c) -> p h c", h=H),
            )

        # skip connection F = xTi^T @ Ws + bs  (i-partitioned, kept open)
        F_ps = f_pool.tile([P, D], f32)
        for kc in range(2):
            mm(F_ps, pk[:, ds(OFF_XTI + kc * 128, 128)], w_ap(OFF_WS, kc, 0, 256),
               start=(kc == 0), stop=(kc == 1 and not has_bias))
        if has_bias:
            mm(F_ps, ones_row[:, :P], b_row(3, 0, 256), start=False, stop=True)

        # QK scores + mask -> qk_sb [128 j, cj, 8 h, 128 i] bf16 (x16 scaled)
        qk_sb = singles.tile([P, 2, H, P], bf16)
        for cj in range(2):
            qk_ps = proj_ps.tile([P, H, P], f32, tag="proj")
            for h in range(H):
                mm(qk_ps[:, h, :], KT[:, h, ds(cj * 128, 128)],
                   QTi[:, h, :], start=True, stop=True)
            nc.vector.tensor_tensor(
                out=qk_sb[:, cj, :, :],
                in0=qk_ps,
                in1=pk[:, ds(OFF_MB + cj * 128, 128)]
                .unsqueeze(1).broadcast_to([P, H, P]),
                op=ADD,
            )

        # u_blk: block-diag fp8 [128 (ii,de), 64 pr, 16 (ii,h)]
        u_blk = singles.tile([P, 64, 16], fp8)
        nc.gpsimd.memset(u_blk, 0.0)
        for hh in range(2):
            u_ps = proj_ps.tile([DE, 4, P], f32, tag="proj")
            for hm in range(4):
                h = hh * 4 + hm
                mm(u_ps[:, hm, :], pk[ds(0, 32), ds(OFF_WETS + h * 64, 64)],
                   QTi[:, h, :], start=True, stop=True)
            upv = u_ps.rearrange("p hm (pr ii) -> p pr ii hm", ii=2)
            for ii in range(2):
                dst = u_blk[ds(ii * DE, DE), :, ds(ii * 8 + hh * 4, 4)]
                if ii == 0:
                    nc.vector.tensor_copy(out=dst, in_=upv[:, :, ii, :])
                else:
                    nc.scalar.activation(out=dst, in_=upv[:, :, ii, :],
                                         func=AF.Copy)

        import os as _os
        _BI = int(_os.environ.get("BISECT", "0"))

        def _emit_out(src_ap):
            t_dbg = singles.tile([P, D], f32, tag="dbg", name="dbg_out")
            nc.vector.memset(t_dbg, 0.0)
            pp = src_ap.partition_size()
            dims = list(src_ap.shape[1:])
            nfree = 1
            for s in dims:
                nfree *= s
            dst = t_dbg[ds(0, pp), ds(0, nfree)]
            if len(dims) == 2:
                dst = dst.rearrange("p (a b) -> p a b", a=dims[0])
            elif len(dims) == 3:
                dst = dst.rearrange("p (a b c) -> p a b c", a=dims[0], b=dims[1])
            nc.vector.tensor_copy(out=dst, in_=src_ap)
            nc.sync.dma_start(out=out[:, :], in_=t_dbg)

        if _BI == 1:
            _emit_out(qk_sb[:, 0, 0:2, :])
            proj_ctx.close()
            return nc
        if _BI == 8:
            ub32 = singles.tile([P, 16, 16], f32)
            nc.vector.tensor_copy(out=ub32, in_=u_blk[:, 0:16, :])
            _emit_out(ub32)
            proj_ctx.close()
            return nc
        if _BI == 7:
            _emit_out(V_sb[:, 0, 0:7, :])
            proj_ctx.close()
            return nc

        proj_ctx.close()
        stream_ctx = ExitStack()
        qe_pool = stream_ctx.enter_context(
            tc.tile_pool(name="qe", bufs=2, space="PSUM")
        )

        # ---------------- edge stream ----------------
        OV = ov_pool.tile([P, H * 33], f32)  # [128 i, (h, 33)]
        agg_ps = [
            agg_pool.tile([P, 32, 16], f32, tag=f"agg{t}", name=f"agg{t}")
            for t in range(2)
        ]
        al_t = [
            singles.tile([P, H, P], bf16, tag=f"al_{cj}", name=f"al_{cj}")
            for cj in range(2)
        ]

        # Tile may reorder independent PE ops, so cross-cj accumulations
        # cannot rely on a start=True first writer arriving first: zero the
        # accumulator regions with explicit rank-1 matmuls (order-safe).
        zrow = singles.tile([1, 512], bf16)
        nc.vector.memset(zrow, 0.0)

        def zero_mm(out_ap, m, n):
            mm(out_ap, zrow[:, :m], zrow[:, :n], start=True, stop=False)

        for t in range(2):
            zero_mm(agg_ps[t].rearrange("p a b -> p (a b)"), P, 512)
        zero_mm(OV, P, H * 33)

        def emit_agg(iq, cj):
            al = al_t[cj]
            half = iq // 2
            for prl in range(16):
                pr = iq * 16 + prl
                mm(agg_ps[half][:, pr - half * 32, :],
                   en_sb[:, cj, ds(pr * 128, 128)],
                   al.rearrange("p h i -> p i h")[:, ds(pr * 2, 2), :],
                   start=False, stop=(cj == 1))

        def emit_outv(ihalf, cj):
            al = al_t[cj]
            for h in range(H):
                mm(OV[ds(ihalf * 64, 64), ds(h * 33, 33)],
                   al[:, h, ds(ihalf * 64, 64)], V_sb[:, cj, h, :],
                   start=False, stop=False)

        # agg_sb [64 de, 128 i, 8 h] bf16; per-half epilogue state
        agg_sb = singles.tile([DE, P, H], bf16)
        av = agg_sb.rearrange("p (pr ii) h -> p pr ii h", ii=2)
        ovv = OV.rearrange("p (h c) -> p h c", c=33)
        den = singles.tile([P, H], f32)
        outp = singles.tile([P, D], f32)
        opv = outp.rearrange("p (h c) -> p h c", c=32)

        def finish_half(hf):
            # extract agg diagonal for this i-half
            for ii in range(2):
                nc.vector.tensor_copy(
                    out=av[:, ds(hf * 32, 32), ii, :],
                    in_=agg_ps[hf][ds(ii * DE, DE), :, ds(ii * 8, 8)],
                )
            # out_e: OV[i, h, :32] += agg[i, h, :] @ We_h
            io = ds(hf * 64, 64)
            for h in range(H):
                mm(OV[io, ds(h * 33, 32)],
                   agg_sb[:, io, h],
                   pk[ds(0, DE), ds(OFF_WE + h * 32, 32)],
                   start=False, stop=(h == H - 1))
            # normalize + skip-add + store (i-partitioned, no transposes)
            nc.vector.tensor_scalar_add(out=den[io, :], in0=ovv[io, :, 32],
                                        scalar1=1e-30)
            nc.vector.reciprocal(out=den[io, :], in_=den[io, :])
            nc.vector.tensor_tensor(
                out=opv[io, :, :], in0=ovv[io, :, 0:32],
                in1=den[io, :].unsqueeze(2).broadcast_to([64, H, 32]), op=MUL,
            )
            nc.vector.tensor_tensor(out=outp[io, :], in0=outp[io, :],
                                    in1=F_ps[io, :], op=ADD)
            nc.sync.dma_start(out=out[io, :], in_=outp[io, :])

        for cj in range(2):
            for iq in range(4):
                qe = qe_pool.tile([P, H, 32], f32)
                qe_v = qe.rearrange("p h i -> p i h")
                for prl in range(16):
                    pr = iq * 16 + prl
                    mm(qe_v[:, ds(prl * 2, 2), :],
                       eT2_sb[:, cj, ds(pr * 128, 128)],
                       u_blk[:, pr, :],
                       start=True, stop=True)
                s_sum = fin_pool.tile([P, H, 32], f32, tag="s_sum")
                nc.vector.tensor_tensor(
                    out=s_sum, in0=qe,
                    in1=qk_sb[:, cj, :, ds(iq * 32, 32)],
                    op=ADD,
                )
                if _BI == 2 and cj == 0 and iq == 0:
                    _emit_out(qe)
                    stream_ctx.close()
                    return nc
                nc.scalar.activation(out=al_t[cj][:, :, ds(iq * 32, 32)],
                                     in_=s_sum, func=AF.Exp, scale=0.0625)
                if _BI == 3 and cj == 0 and iq == 0:
                    _emit_out(al_t[0][:, :, 0:32])
                    stream_ctx.close()
                    return nc
                # software-pipeline: agg of the previous iq
                if iq >= 1:
                    emit_agg(iq - 1, cj)
                if cj == 1 and iq == 2:
                    # agg half 0 (iq 0,1 x both cj) is complete: finish the
                    # lower i-half while iq3/agg2/agg3 still stream.
                    emit_outv(0, 1)
                    finish_half(0)
            emit_agg(3, cj)
            if cj == 0:
                emit_outv(0, 0)
                emit_outv(1, 0)
            else:
                emit_outv(1, 1)
                finish_half(1)

        stream_ctx.close()

    return nc


def _split_multi_waits(nc):
    """Walrus TRN2 codegen encodes at most ONE sync wait per engine
    instruction; Tile's wait assignment is not transitively minimal and
    emits 2-3.  Hoist all but one wait onto same-engine no-ops."""
    import concourse.mybir as mybir

    for fn in nc.m.functions:
        for blk in fn.blocks:
            new_insts = []
            for inst in blk.instructions:
                si = inst.sync_info
                if (
                    si is not None
                    and len(si.on_wait) > 1
                    and type(inst).__name__ != "InstEventSemaphore"
                ):
                    waits = list(si.on_wait)
                    for k, w in enumerate(waits[:-1]):
                        nop = mybir.InstNoOp(name=f"{inst.name}-sw{k}", ins=[], outs=[])
                        nop.engine = inst.engine
                        nop.sync_info = mybir.SyncInfo(on_wait=[w], on_update=[])
                        nc.register_instruction(nop)
                        new_insts.append(nop)
                    inst.sync_info = mybir.SyncInfo(
                        on_wait=[waits[-1]], on_update=list(si.on_update)
                    )
                new_insts.append(inst)
            blk.instructions = new_insts


def _get_program(has_bias=False):
    if has_bias not in _PROGRAM:
        nc = _build_program(has_bias)
        _split_multi_waits(nc)
        _PROGRAM[has_bias] = nc
    return _PROGRAM[has_bias]


def _prep_weights(W_q, b_q, W_k, b_k, W_v, b_v, W_e, W_s, b_s):
    """Shared (per-run) weight block of the packed buffer, bf16."""
    bf = ml_dtypes.bfloat16
    scale = np.float32(1.0 / np.sqrt(C))
    s16 = np.float32(16.0)

    def w2(w):  # (256,256) -> [128, 512] (kc-major row chunks)
        w = np.asarray(w, np.float32)
        return np.concatenate([w[0:128, :], w[128:256, :]], axis=1)

    wq = w2(np.asarray(W_q, np.float32) * s16)
    wk = w2(np.asarray(W_k, np.float32) * scale)
    wv = w2(W_v)
    ws = w2(W_s)
    wets = np.asarray(W_e, np.float32).T * scale  # [256 d', 64]
    # [32 c, 8 h, 64 de] -> rows 0-31 of a [128, 512] slot
    wets2 = np.zeros((P, 512), np.float32)
    wets2[0:32, :] = wets.reshape(H, 32, DE).transpose(1, 0, 2).reshape(32, 512)
    we = np.zeros((P, 256), np.float32)
    we[0:DE, :] = np.asarray(W_e, np.float32)
    biases = np.zeros((P, 1024 + 512), np.float32)
    biases[0, 0:256] = np.asarray(b_q, np.float32) * s16
    biases[0, 256:512] = np.asarray(b_k, np.float32) * scale
    biases[0, 512:768] = np.asarray(b_v, np.float32)
    biases[0, 768:1024] = np.asarray(b_s, np.float32)
    biases[0, 1024:1536] = 1.0
    blk = (wq, wk, wv, ws)
    has_bias = bool(
        np.any(np.asarray(b_q)) or np.any(np.asarray(b_k))
        or np.any(np.asarray(b_v)) or np.any(np.asarray(b_s))
    )
    return ([b.astype(bf) for b in blk], wets2.astype(bf), we.astype(bf),
            biases.astype(bf), has_bias)


def _prep_core_inputs(c, x, edge_attr, attn_mask, wblk):
    bf = ml_dtypes.bfloat16
    f8 = ml_dtypes.float8_e4m3
    weights, wets2, we, biases, _has_bias = wblk
    b, ih = c // 2, c % 2
    i0 = ih * P

    xb = np.asarray(x[b], np.float32)
    xT = np.concatenate([xb.T[0:128, :], xb.T[128:256, :]], axis=1)  # [128,512]
    xi = xb[i0 : i0 + P].T
    xTi = np.concatenate([xi[0:128, :], xi[128:256, :]], axis=1)  # [128,256]
    mb = (np.asarray(attn_mask[b, i0 : i0 + P]).T.astype(np.float32) * 800.0
          - 800.0)  # [256 j, 128 i]
    mb2 = np.concatenate([mb[0:128, :], mb[128:256, :]], axis=1)  # [128, 256]

    wq, wk, wv, ws = weights
    packed = np.empty((P, PACK_COLS), bf)
    packed[:, OFF_WQ : OFF_WQ + 512] = wq
    packed[:, OFF_WK : OFF_WK + 512] = wk
    packed[:, OFF_XT : OFF_XT + 512] = xT.astype(bf)
    packed[:, OFF_XTI : OFF_XTI + 256] = xTi.astype(bf)
    packed[:, OFF_WETS : OFF_WETS + 512] = wets2
    packed[:, OFF_MB : OFF_MB + 256] = mb2.astype(bf)
    packed[:, OFF_WV : OFF_WV + 512] = wv
    packed[:, OFF_WS : OFF_WS + 512] = ws
    packed[:, OFF_WE : OFF_WE + 256] = we
    packed[:, OFF_BIAS:] = biases

    ec = np.asarray(edge_attr[b, i0 : i0 + P], np.float32)  # [128 i, 256 j, 64]
    en_h = (ec.transpose(1, 0, 2).reshape(2, P, P * DE)).astype(bf)
    eT2_h = np.ascontiguousarray(
        ec.reshape(64, 2, 2, 128, DE).transpose(2, 1, 4, 0, 3)
    ).reshape(2, P, 64 * P).astype(f8)
    return {
        "packed": packed,
        "en": np.ascontiguousarray(en_h),
        "eT2": np.ascontiguousarray(eT2_h),
    }


def kernel(x, edge_attr, attn_mask, W_q, b_q, W_k, b_k, W_v, b_v, W_e, W_s, b_s):
    from concourse.bass_utils import run_bass_kernel_spmd

    x = np.asarray(x, dtype=np.float32)
    edge_attr = np.asarray(edge_attr, dtype=np.float32)
    attn_mask = np.asarray(attn_mask)
    wblk = _prep_weights(W_q, b_q, W_k, b_k, W_v, b_v, W_e, W_s, b_s)

    nc = _get_program(wblk[-1])
    in_maps = [
        _prep_core_inputs(c, x, edge_attr, attn_mask, wblk) for c in range(NCORES)
    ]
    res = run_bass_kernel_spmd(nc, in_maps, core_ids=list(range(NCORES)))
    outv = np.empty((B, N, D), dtype=np.float32)
    for c in range(NCORES):
        b, ih = c // 2, c % 2
        outv[b, ih * P : (ih + 1) * P] = np.asarray(res.results[c]["out"])
    return outv
